# revision 3
# baseline (speedup 1.0000x reference)
"""Bidirectional RNN (tanh) Trainium2 Bass kernel.

Problem: x [64, 1024, 512] f32; per direction d:
    h_t = tanh(x_t @ Wih^T + h_{t-1} @ Whh^T + bih + bhh)
    y_t = h_t @ fcW^T + fcb
backward direction consumes time-reversed x and its outputs are NOT
re-flipped. Returns (y [64,1024,1024] = concat(fy, by), h_out [2,64,512]).

Sharding: 8 cores = 2 directions x 4 batch groups of 16 sequences.
Each core runs one direction's full 1024-step recurrence for 16 seqs.

Per-core layout ("folded transpose"): hidden dim on partitions.
    h^T tile: [128, 4, 16]  with h[b, kc*128+p] = tile[p, kc, b]
    Hbuf/Ubuf chunk: [128, 4, CH*16] free idx = (kc, t*16+b)
Recurrence step (PE): 1 identity-matmul injects u_t (precomputed
x@Wih^T + b) into PSUM (sets has_written), then 16 accumulating
128x128 matmuls stream Whh; ACT tanh evacuates PSUM -> SBUF h^T.
U precompute and the FC output matmul are batched over chunks and fill
the PE gaps left by the serial tanh dependency.

Matmul operands are bf16 (PSUM accumulation stays fp32): fp32 matmuls
lower to 2 HW passes (fp32_mode=LOW_HIGH) and the stationary-weight
load path runs at half rate without FWL, so bf16 roughly quadruples
recurrence throughput at ~4e-3 relative error (tanh is contractive).
"""

import ml_dtypes
import numpy as np

import concourse.bass as bass
import concourse.mybir as mybir
import concourse.tile as tile
from concourse import bacc
from concourse.bass_utils import run_bass_kernel_spmd

P = 128
B = 16          # batch rows per core
L = 1024        # timesteps
I = 512         # input features
H = 512         # hidden
KC = H // P     # 4 hidden-dim chunks
CH = 64         # timesteps per chunk
NCH = L // CH
F32 = mybir.dt.float32
BF16 = mybir.dt.bfloat16
NP_BF16 = ml_dtypes.bfloat16

_CACHE = {}


def _build():
    nc = bacc.Bacc("TRN2", target_bir_lowering=False, debug=False,
                   enable_asserts=True)

    xT = nc.dram_tensor("xT", [I, L * B], BF16, kind="ExternalInput")
    wstat = nc.dram_tensor("wstat", [P, KC * H], BF16, kind="ExternalInput")
    wih = nc.dram_tensor("wih", [P, KC * H], BF16, kind="ExternalInput")
    fcwt = nc.dram_tensor("fcwt", [P, KC * H], BF16, kind="ExternalInput")
    bias_u = nc.dram_tensor("bias_u", [P, KC], F32, kind="ExternalInput")
    fcb = nc.dram_tensor("fcb", [P, H], F32, kind="ExternalInput")
    ident = nc.dram_tensor("ident", [P, P], BF16, kind="ExternalInput")
    h0 = nc.dram_tensor("h0", [P, KC * B], BF16, kind="ExternalInput")

    y = nc.dram_tensor("y", [L, B, H], F32, kind="ExternalOutput")
    hn = nc.dram_tensor("hn", [P, KC * B], BF16, kind="ExternalOutput")

    with tile.TileContext(nc) as tc:
        with (
            tc.tile_pool(name="const", bufs=1) as const,
            tc.tile_pool(name="hbuf", bufs=2) as hbufp,
            tc.tile_pool(name="ubuf", bufs=2) as ubufp,
            tc.tile_pool(name="xsb", bufs=8) as xsbp,
            tc.tile_pool(name="ysb", bufs=4) as ysbp,
            tc.tile_pool(name="ph", bufs=2, space="PSUM") as php,
            tc.tile_pool(name="pu", bufs=2, space="PSUM") as pup,
            tc.tile_pool(name="py", bufs=2, space="PSUM") as pyp,
        ):
            wstat_sb = const.tile([P, KC * H], BF16, tag="wstat")
            wih_sb = const.tile([P, KC * H], BF16, tag="wih")
            fcw_sb = const.tile([P, KC * H], BF16, tag="fcw")
            fcb_sb = const.tile([P, H], F32, tag="fcb")
            ident_sb = const.tile([P, P], BF16, tag="ident")
            biasu_sb = const.tile([P, KC], F32, tag="biasu")
            h0_sb = const.tile([P, KC, B], BF16, tag="h0")
            nc.sync.dma_start(wstat_sb[:], wstat.ap())
            nc.sync.dma_start(wih_sb[:], wih.ap())
            nc.sync.dma_start(fcw_sb[:], fcwt.ap())
            nc.sync.dma_start(fcb_sb[:], fcb.ap())
            nc.sync.dma_start(ident_sb[:], ident.ap())
            nc.sync.dma_start(biasu_sb[:], bias_u.ap())
            nc.sync.dma_start(h0_sb[:], h0.ap())

            xT_ap = xT.ap()
            y_ap = y.ap()

            TH = CH * B // 512  # 512-column groups per chunk (2 for CH=64)

            def emit_u(ch, ubuf_t):
                for th in range(TH):
                    col0 = (ch * CH) * B + th * 512
                    xts = []
                    for kc in range(KC):
                        xt = xsbp.tile([P, 512], BF16, tag="xsb")
                        nc.sync.dma_start(
                            xt[:], xT_ap[kc * P:(kc + 1) * P, col0:col0 + 512])
                        xts.append(xt)
                    for jc in range(KC):
                        pu_t = pup.tile([P, 512], F32, tag="pu")
                        for kc in range(KC):
                            nc.tensor.matmul(
                                pu_t[:],
                                wih_sb[:, kc * H + jc * P: kc * H + (jc + 1) * P],
                                xts[kc][:],
                                start=(kc == 0), stop=(kc == KC - 1))
                        nc.vector.tensor_scalar_add(
                            ubuf_t[:, jc, th * 512:(th + 1) * 512],
                            pu_t[:], biasu_sb[:, jc:jc + 1])

            def emit_fc(ch, hbuf_t):
                for rt in range(CH * B // P):  # 8 row-tiles of 128 rows
                    py_t = pyp.tile([P, H], F32, tag="py")
                    for kc in range(KC):
                        nc.tensor.matmul(
                            py_t[:],
                            hbuf_t[:, kc, rt * P:(rt + 1) * P],
                            fcw_sb[:, kc * H:(kc + 1) * H],
                            start=(kc == 0), stop=(kc == KC - 1))
                    yt = ysbp.tile([P, H], F32, tag="ysb")
                    nc.vector.tensor_add(yt[:], py_t[:], fcb_sb[:])
                    t0 = ch * CH + rt * (P // B)
                    nc.sync.dma_start(y_ap[t0:t0 + P // B], yt[:])

            ubuf_cur = ubufp.tile([P, KC, CH * B], BF16, tag="ubuf")
            emit_u(0, ubuf_cur)

            hbuf_prev = None
            for ch in range(NCH):
                hbuf_cur = hbufp.tile([P, KC, CH * B], BF16, tag="hbuf")
                for t in range(CH):
                    if t == 0:
                        if ch == 0:
                            hprev = h0_sb
                            tp = 0
                        else:
                            hprev = hbuf_prev
                            tp = CH - 1
                    else:
                        hprev = hbuf_cur
                        tp = t - 1
                    ph_t = php.tile([P, KC, B], F32, tag="ph")
                    nc.tensor.matmul(ph_t[:], ident_sb[:],
                                     ubuf_cur[:, :, t * B:(t + 1) * B],
                                     start=True, stop=False)
                    for mc in range(KC):
                        for kc in range(KC):
                            nc.tensor.matmul(
                                ph_t[:, mc],
                                wstat_sb[:, kc * H + mc * P: kc * H + (mc + 1) * P],
                                hprev[:, kc, tp * B:(tp + 1) * B],
                                start=False,
                                stop=(mc == KC - 1 and kc == KC - 1))
                    nc.scalar.activation(hbuf_cur[:, :, t * B:(t + 1) * B],
                                         ph_t[:],
                                         mybir.ActivationFunctionType.Tanh)

                emit_fc(ch, hbuf_cur)
                if ch + 1 < NCH:
                    ubuf_next = ubufp.tile([P, KC, CH * B], BF16, tag="ubuf")
                    emit_u(ch + 1, ubuf_next)
                    ubuf_cur = ubuf_next
                hbuf_prev = hbuf_cur

            nc.sync.dma_start(
                hn.ap(), hbuf_prev[:, :, (CH - 1) * B:CH * B])

    nc.compile()
    return nc


def _prep_core_inputs(x_g, h0_g, Wih, Whh, bih, bhh, fcW, fcb_v, ident):
    """x_g [B, L, I] (already time-flipped for bw); h0_g [B, H]."""
    xT = np.ascontiguousarray(
        x_g.transpose(2, 1, 0).reshape(I, L * B)).astype(NP_BF16)

    def stat(w):  # [H, K] -> [P, kc*H + mc*P + q] = w[mc*P+q, kc*P+p]
        return np.ascontiguousarray(
            w.reshape(KC, P, KC, P).transpose(3, 2, 0, 1).reshape(P, KC * H)
        ).astype(NP_BF16)

    wstat = stat(Whh)
    wih_s = stat(Wih)
    fcwt = np.ascontiguousarray(
        fcW.T.reshape(KC, P, H).transpose(1, 0, 2).reshape(P, KC * H)
    ).astype(NP_BF16)
    bias_u = np.ascontiguousarray((bih + bhh).reshape(KC, P).T)
    fcb_b = np.ascontiguousarray(
        np.broadcast_to(fcb_v, (P, H)).astype(np.float32))
    h0T = np.ascontiguousarray(
        h0_g.reshape(B, KC, P).transpose(2, 1, 0).reshape(P, KC * B)
    ).astype(NP_BF16)
    return {
        "xT": xT, "wstat": wstat, "wih": wih_s, "fcwt": fcwt,
        "bias_u": bias_u, "fcb": fcb_b, "ident": ident, "h0": h0T,
    }


def kernel(x, h, fw_Wih, fw_Whh, fw_bih, fw_bhh, fw_fcW, fw_fcb,
           bw_Wih, bw_Whh, bw_bih, bw_bhh, bw_fcW, bw_fcb,
           trace=False):
    x = np.asarray(x, dtype=np.float32)
    h = np.asarray(h, dtype=np.float32)
    args = {k: np.asarray(v, dtype=np.float32) for k, v in {
        "fw_Wih": fw_Wih, "fw_Whh": fw_Whh, "fw_bih": fw_bih,
        "fw_bhh": fw_bhh, "fw_fcW": fw_fcW, "fw_fcb": fw_fcb,
        "bw_Wih": bw_Wih, "bw_Whh": bw_Whh, "bw_bih": bw_bih,
        "bw_bhh": bw_bhh, "bw_fcW": bw_fcW, "bw_fcb": bw_fcb,
    }.items()}

    if "nc" not in _CACHE:
        _CACHE["nc"] = _build()
    nc = _CACHE["nc"]

    ident = np.eye(P, dtype=np.float32).astype(NP_BF16)
    x_flip = x[:, ::-1]
    in_maps = []
    for core in range(8):
        d, g = core % 2, core // 2
        bs = slice(g * B, (g + 1) * B)
        if d == 0:
            in_maps.append(_prep_core_inputs(
                x[bs], h[0, bs], args["fw_Wih"], args["fw_Whh"],
                args["fw_bih"], args["fw_bhh"], args["fw_fcW"],
                args["fw_fcb"], ident))
        else:
            in_maps.append(_prep_core_inputs(
                x_flip[bs], h[1, bs], args["bw_Wih"], args["bw_Whh"],
                args["bw_bih"], args["bw_bhh"], args["bw_fcW"],
                args["bw_fcb"], ident))

    if trace:
        # Profiling is only stable on a single-core run; cores are
        # symmetric SPMD so core 0's exec time is the kernel time.
        res = run_bass_kernel_spmd(nc, in_maps[:1], core_ids=[0], trace=True)
        _CACHE["last_result"] = res
        res = run_bass_kernel_spmd(nc, in_maps, core_ids=list(range(8)))
    else:
        res = run_bass_kernel_spmd(nc, in_maps, core_ids=list(range(8)))
        _CACHE["last_result"] = res

    y = np.empty((64, L, 2 * H), dtype=np.float32)
    h_out = np.empty((2, 64, H), dtype=np.float32)
    for core in range(8):
        d, g = core % 2, core // 2
        bs = slice(g * B, (g + 1) * B)
        yc = res.results[core]["y"]          # [L, B, H] t-major
        y[bs, :, d * H:(d + 1) * H] = yc.transpose(1, 0, 2)
        hnc = res.results[core]["hn"].astype(np.float32)   # [P, KC*B]
        h_out[d, bs, :] = hnc.reshape(P, KC, B).transpose(2, 1, 0).reshape(B, H)
    return y, h_out
